# revision 21
# baseline (speedup 1.0000x reference)
"""Causal self-attention (sparse column mask) on 8 Trainium2 NeuronCores.

Problem: B=8, T=1024, C=512, 8 heads (hd=64).
  q/k/v = x @ W{q,k,v}.T + b;  att = softmax(mask(q k^T / 8));  y = att v
  out = y @ Wp.T + bp
Mask: causal lower-triangle, minus every column j with j % 25 == 24.

Sharding: pure data-parallel over batch - core b computes batch element b.

v2 design (fp16 matmul operands, f32 PSUM). Same math as v1; restructured
schedule so the PE stream is dense from ~2us on:
  - k-outer projection start: the first q/k projections consume x/W DMA
    chunks as they arrive instead of waiting for the full tensors.
  - QK tile pairs run concurrently in the PE array (row groups 0/64); AV and
    the ones-matmul denominators run as concurrent column-group pairs.
  - Fine-grained interleave: qk/av/proj/outproj matmuls are woven so the PE
    never has to wait for ACT exp between QK tiles (st PSUM recycling).
  - Output projection r0..3 only needs the first query half -> runs as filler
    during the second attention chunk; r4..7 are pre-accumulated (m=0..2)
    into PSUM banks freed by the last QK phase, so the tail after the final
    AV is just 4 matmuls + evac + small fp16 DMAs.
  - Output written fp16 (host upcasts); per-row-tile DMA on the gpsimd ring.
"""

import numpy as np

B, T, C = 8, 1024, 512
H = 8
HD = C // H
P = 128
JD = 25  # joined dim; column j masked when j % 25 == 24
N_CORES = 8
NEG = -30.0  # added post-scale; exp(-30) flushes to 0 in fp16

_CACHE = {}


def _build():
    import concourse.bass as bass
    import concourse.mybir as mybir
    import concourse.tile as tile
    from concourse import bacc

    f16 = mybir.dt.float16
    f32 = mybir.dt.float32
    AF = mybir.ActivationFunctionType
    ALU = mybir.AluOpType

    nc = bacc.Bacc("TRN2", target_bir_lowering=False, debug=False)

    KT = C // P  # 4 c_in tiles
    MT = C // P  # 4 c_out tiles (= head pairs)
    RT = T // P  # 8 t tiles

    xT = nc.dram_tensor("xT", [C, T], f16, kind="ExternalInput").ap()
    # wq/wk column-interleaved per c_in row (2KB DMA rows); wv/wp pre-swizzled
    # to their SBUF layout (4KB DMA rows, single transfer each).
    wqk = nc.dram_tensor("wqk", [C, 2 * C], f16, kind="ExternalInput").ap()
    wv_d = nc.dram_tensor("wv_sw", [P, KT * C], f16, kind="ExternalInput").ap()
    wp_d = nc.dram_tensor("wp_sw", [P, KT * C], f16, kind="ExternalInput").ap()
    bq = nc.dram_tensor("bq", [P, C // P], f32, kind="ExternalInput").ap()
    bppb = nc.dram_tensor("bppb", [P, C], f32, kind="ExternalInput").ap()
    ones64 = nc.dram_tensor("ones64", [P, HD], f16, kind="ExternalInput").ap()
    ones_row = nc.dram_tensor("ones_row", [1, P], f16, kind="ExternalInput").ap()
    bpp_row = nc.dram_tensor("bpp_row", [1, C], f16, kind="ExternalInput").ap()
    tri = nc.dram_tensor("tri", [P, P], f16, kind="ExternalInput").ap()
    cmask = nc.dram_tensor("cmask", [P, T // P], f32, kind="ExternalInput").ap()
    out = nc.dram_tensor("out", [T, C], f16, kind="ExternalOutput").ap()

    with tile.TileContext(nc) as tc:
        with (
            tc.tile_pool(name="const", bufs=1) as const,
            tc.tile_pool(name="persist", bufs=1) as persist,
            tc.tile_pool(name="es", bufs=16) as es_pool,
            tc.tile_pool(name="rden", bufs=4) as rden_pool,
            tc.tile_pool(name="ot", bufs=4) as ot_pool,
            tc.tile_pool(name="stp", bufs=2, space="PSUM") as stp,
            tc.tile_pool(name="work", bufs=4, space="PSUM") as work,
        ):
            # ---- input DMAs: chunked, ordered by first consumption ----
            def load(shape, dtype, src, tag, eng):
                t = const.tile(shape, dtype, name=tag, tag=tag)
                eng.dma_start(out=t, in_=src)
                return t

            r3 = lambda a: a.rearrange("(a p) n -> p a n", p=P)  # noqa: E731
            # sync (HWDGE) ring, ordered by first consumption: per-k pairs
            # (xT full-row chunk, wq||wk interleaved chunk) - 2KB DMA rows -
            # then the pre-swizzled wv (4KB rows, one transfer).
            # k=0 split small so the very first projection matmul starts ~2us
            # earlier (less data to wait for through the DMA-queue ramp).
            xT_c, wqk_c = [None] * KT, [None] * KT
            x0a = load([P, 1, 512], f16, r3(xT)[:, 0:1, 0:512], "xT0a", nc.sync)
            wq0 = load([P, 1, C], f16, r3(wqk)[:, 0:1, 0:C], "wq0", nc.sync)
            x0b = load([P, 1, 512], f16, r3(xT)[:, 0:1, 512:1024], "xT0b", nc.sync)
            wk0 = load([P, 1, C], f16, r3(wqk)[:, 0:1, C : 2 * C], "wk0", nc.sync)
            for k in range(1, KT):
                xT_c[k] = load([P, 1, T], f16, r3(xT)[:, k : k + 1, :], f"xT{k}", nc.sync)
                wqk_c[k] = load(
                    [P, 1, 2 * C], f16, r3(wqk)[:, k : k + 1, :], f"wqk{k}", nc.sync
                )
            wv_s = load([P, KT * C], f16, wv_d, "wv", nc.sync)
            # small consts on the scalar (ACT) ring: done before the first exp
            tri_s = load([P, P], f16, tri, "tri", nc.scalar)
            cmask_s = load([P, T // P], f32, cmask, "cmask", nc.scalar)
            bq_s = load([P, C // P], f32, bq, "bq", nc.scalar)
            ones64_s = load([P, HD], f16, ones64, "ones64", nc.scalar)
            ones_row_s = load([1, P], f16, ones_row, "ones_row", nc.scalar)
            bpp_row_s = load([1, C], f16, bpp_row, "bpp_row", nc.scalar)
            # late-needed bulk on the gpsimd software-DGE ring
            bppb_s = load([P, C], f32, bppb, "bppb", nc.gpsimd)
            wp_s = load([P, KT * C], f16, wp_d, "wp", nc.gpsimd)

            def xt(k, h):
                if k == 0:
                    return (x0a if h == 0 else x0b)[:, 0, :]
                return xT_c[k][:, 0, 512 * h : 512 * (h + 1)]

            def xts(k, c0, w):  # columns [c0, c0+w) of xT chunk k
                if k == 0:
                    t = x0a if c0 < 512 else x0b
                    c0 = c0 % 512
                    return t[:, 0, c0 : c0 + w]
                return xT_c[k][:, 0, c0 : c0 + w]

            def wq_sl(k, m):
                if k == 0:
                    return wq0[:, 0, P * m : P * (m + 1)]
                return wqk_c[k][:, 0, P * m : P * (m + 1)]

            def wk_sl(k, m):
                if k == 0:
                    return wk0[:, 0, P * m : P * (m + 1)]
                return wqk_c[k][:, 0, C + P * m : C + P * (m + 1)]

            qT_t = [persist.tile([P, T], f16, name=f"qT{m}", tag=f"qT{m}") for m in range(MT)]
            kT_t = [persist.tile([P, T], f16, name=f"kT{m}", tag=f"kT{m}") for m in range(MT)]
            v_t = [persist.tile([P, C], f16, name=f"v{r}", tag=f"v{r}") for r in range(RT)]
            yn_t = [persist.tile([P, T], f16, name=f"yn{m}", tag=f"yn{m}") for m in range(MT)]

            # broadcast lower-triangle tile across both heads of an es tile
            tri_b = bass.AP(
                tensor=tri_s.tensor,
                offset=tri_s.offset,
                ap=[list(tri_s.ap[0]), [0, 2], list(tri_s.ap[1])],
            )

            # ---- emission helpers ----
            def proj_qk_m(m, h):
                """q and k projections for pair m, query half h (k-outer)."""
                qs = work.tile([P, 512], f32, name="qps", tag="wk")
                ks = work.tile([P, 512], f32, name="kps", tag="wk")
                for k in range(KT):
                    nc.tensor.matmul(
                        qs,
                        lhsT=wq_sl(k, m),
                        rhs=xt(k, h),
                        start=(k == 0),
                        stop=(k == KT - 1),
                    )
                    nc.tensor.matmul(
                        ks,
                        lhsT=wk_sl(k, m),
                        rhs=xt(k, h),
                        start=(k == 0),
                        stop=(k == KT - 1),
                    )
                sl = slice(512 * h, 512 * (h + 1))
                nc.vector.tensor_scalar_add(qT_t[m][:, sl], qs, bq_s[:, m : m + 1])
                nc.vector.tensor_copy(kT_t[m][:, sl], ks)

            def proj_v(r):
                pv = work.tile([P, C], f32, name="pv", tag="wk")
                for k in range(KT):
                    nc.tensor.matmul(
                        pv,
                        lhsT=xts(k, P * r, P),
                        rhs=wv_s[:, C * k : C * (k + 1)],
                        start=(k == 0),
                        stop=(k == KT - 1),
                    )
                nc.vector.tensor_copy(v_t[r], pv)

            es_t = {}

            def qk2(ic, p, jp):
                """two QK key-tiles (J=2jp, 2jp+1) + exp (+ causal tri)."""
                for J in (2 * jp, 2 * jp + 1):
                    if J >= 4 * (ic + 1):
                        continue
                    i0 = max(512 * ic, P * J)
                    w = 512 * (ic + 1) - i0
                    st = stp.tile([P, 2, 512], f32, name="st", tag="st")
                    for h in range(2):
                        nc.tensor.matmul(
                            st[:, h, :w],
                            lhsT=kT_t[p][64 * h : 64 * (h + 1), P * J : P * (J + 1)],
                            rhs=qT_t[p][64 * h : 64 * (h + 1), i0 : i0 + w],
                            start=True,
                            stop=True,
                            tile_position=(64 * h, 0),
                        )
                    es = es_pool.tile([P, 2, 512], f16, name="es", tag="es")
                    es_t[(ic, p, J)] = es
                    nc.scalar.activation(
                        es[:, :, :w],
                        st[:, :, :w],
                        AF.Exp,
                        bias=cmask_s[:, J : J + 1],
                        scale=0.125,
                    )
                    if P * J >= 512 * ic:  # diagonal: zero the causal triangle
                        nc.vector.tensor_tensor(
                            out=es[:, :, :P], in0=es[:, :, :P], in1=tri_b, op=ALU.mult
                        )

            av_ps = {}

            def av2(ic, p, jp):
                """two AV+den key-tiles for (ic, p)."""
                nJ = 4 * (ic + 1)
                if jp == 0 and (ic, p) not in av_ps:
                    av_ps[(ic, p)] = (
                        work.tile([P, 512], f32, name="av", tag="wk"),
                        work.tile([P, 512], f32, name="den", tag="wk"),
                    )
                av, den = av_ps[(ic, p)]
                for J in (2 * jp, 2 * jp + 1):
                    if J >= nJ:
                        continue
                    i0 = max(512 * ic, P * J)
                    w = 512 * (ic + 1) - i0
                    io = i0 - 512 * ic
                    first, last = J == 0, J == nJ - 1
                    es = es_t.pop((ic, p, J))
                    for h in range(2):
                        nc.tensor.matmul(
                            av[64 * h : 64 * (h + 1), io : io + w],
                            lhsT=v_t[J][:, P * p + 64 * h : P * p + 64 * (h + 1)],
                            rhs=es[:, h, :w],
                            start=first,
                            stop=last,
                            tile_position=(0, 64 * h),
                        )
                        nc.tensor.matmul(
                            den[64 * h : 64 * (h + 1), io : io + w],
                            lhsT=ones64_s,
                            rhs=es[:, h, :w],
                            start=first,
                            stop=last,
                            tile_position=(0, 64 * h),
                        )

            def rden_mul(ic, p):
                av, den = av_ps.pop((ic, p))
                rden = rden_pool.tile([P, 512], f32, name="rden", tag="rden")
                nc.vector.reciprocal_approx_fast(out=rden, in_=den)
                nc.vector.tensor_mul(yn_t[p][:, 512 * ic : 512 * (ic + 1)], av, rden)

            def av4(ic, p):
                av2(ic, p, 0)
                av2(ic, p, 1)
                rden_mul(ic, p)

            def op_emit(r, po, m0, m1, bias_mm=False):
                """outproj matmuls m0..m1 for row-tile r into po."""
                for m in range(m0, m1 + 1):
                    nc.tensor.matmul(
                        po,
                        lhsT=yn_t[m][:, P * r : P * (r + 1)],
                        rhs=wp_s[:, C * m : C * (m + 1)],
                        start=(m == 0),
                        stop=(m == MT - 1),
                    )
                if bias_mm:  # rank-1 bias add folded into the PSUM group
                    nc.tensor.matmul(
                        po, lhsT=ones_row_s, rhs=bpp_row_s, start=False, stop=False
                    )

            def op_dma(r, ot):
                nc.sync.dma_start(out=out[P * r : P * (r + 1), :], in_=ot)

            def op_full(r):
                po = work.tile([P, C], f32, name=f"po{r}", tag="wk")
                op_emit(r, po, 0, MT - 1)
                ot = ot_pool.tile([P, C], f16, name="ot", tag="ot")
                nc.vector.tensor_tensor(out=ot, in0=po, in1=bppb_s, op=ALU.add)
                op_dma(r, ot)

            # ================= emission schedule =================
            # A: projections h0 woven with the first QK tiles
            proj_qk_m(0, 0)
            qk2(0, 0, 0)
            proj_qk_m(1, 0)
            qk2(0, 0, 1)
            proj_qk_m(2, 0)
            qk2(0, 1, 0)
            proj_qk_m(3, 0)
            qk2(0, 1, 1)
            # B: ic=0 attention + v + h1 projections
            proj_v(0)
            proj_v(1)
            qk2(0, 2, 0)
            proj_v(2)
            proj_v(3)
            qk2(0, 2, 1)
            av4(0, 0)
            qk2(0, 3, 0)
            proj_qk_m(0, 1)
            qk2(0, 3, 1)
            av4(0, 1)
            proj_qk_m(1, 1)
            av4(0, 2)
            proj_qk_m(2, 1)
            av4(0, 3)
            proj_qk_m(3, 1)
            # C: ic=1 attention; av lags qk by one phase; outproj r0..3 and the
            # remaining v tiles fill the gaps.
            qk2(1, 0, 0)
            op_full(0)
            qk2(1, 0, 1)
            op_full(1)
            qk2(1, 0, 2)
            proj_v(4)
            proj_v(5)
            qk2(1, 0, 3)
            proj_v(6)
            proj_v(7)
            for p in (1, 2, 3):
                for jp in range(4):
                    qk2(1, p, jp)
                    av2(1, p - 1, jp)
                    if p == 1 and jp == 0:
                        op_full(2)
                    if p == 1 and jp == 1:
                        op_full(3)
                rden_mul(1, p - 1)
            # tail: av(1,3) accumulates into one st-pool tile (avdn[:,0]=av,
            # avdn[:,1]=den), freeing four independent work-pool banks for the
            # pre-accumulated outproj r4..7 (m=0..2 plus the rank-1 bias), so
            # the final matmuls and evacuations don't serialize on shared
            # PSUM tiles. The (1,3) normalization runs in 128-column chunks
            # that finalize as soon as their last contributing key-tile lands.
            avdn = stp.tile([P, 2, 512], f32, name="avdn", tag="st")
            av_ps[(1, 3)] = (avdn[:, 0, :], avdn[:, 1, :])
            po_hi = {
                r: work.tile([P, C], f32, name=f"po{r}", tag="wk") for r in range(4, 8)
            }
            av2(1, 3, 0)
            op_emit(4, po_hi[4], 0, 2, bias_mm=True)
            op_emit(5, po_hi[5], 0, 2, bias_mm=True)
            av2(1, 3, 1)
            op_emit(6, po_hi[6], 0, 2, bias_mm=True)
            op_emit(7, po_hi[7], 0, 2, bias_mm=True)
            av2(1, 3, 2)

            av13, den13 = av_ps[(1, 3)]
            rden13 = rden_pool.tile([P, 512], f32, name="rden", tag="rden")

            def norm_chunk(c0, c1):  # columns [128*c0, 128*c1) of the ic=1 half
                sl = slice(P * c0, P * c1)
                nc.vector.reciprocal_approx_fast(out=rden13[:, sl], in_=den13[:, sl])
                nc.vector.tensor_mul(
                    yn_t[3][:, 512 + P * c0 : 512 + P * c1], av13[:, sl], rden13[:, sl]
                )

            norm_chunk(0, 2)  # queries 512..767: all key-tiles J0..J5 done
            op_emit(4, po_hi[4], 3, 3)
            op_emit(5, po_hi[5], 3, 3)
            av2(1, 3, 3)
            norm_chunk(2, 4)
            op_emit(6, po_hi[6], 3, 3)
            op_emit(7, po_hi[7], 3, 3)
            # evacuations split across ACT (idle by now) and DVE
            for r in range(4, 8):
                ot = ot_pool.tile([P, C], f16, name="ot", tag="ot")
                if r % 2 == 0:
                    nc.scalar.activation(ot, po_hi[r], AF.Copy)
                else:
                    nc.vector.tensor_copy(ot, po_hi[r])
                op_dma(r, ot)

    nc.compile()
    return nc


def _prep_inputs(x, Wq, bq, Wk, bk, Wv, bv, Wp, bp):
    """Host-side prep: transposes, bias folding, mask tables. Returns in_maps."""
    f16 = np.float16
    KT = C // P
    wqT = np.ascontiguousarray(Wq.T).astype(f16)
    wkT = np.ascontiguousarray(Wk.T).astype(f16)
    wvT = np.ascontiguousarray(Wv.T).astype(f16)
    wpT = np.ascontiguousarray(Wp.T).astype(f16)
    # wq/wk column-interleaved per c_in (2KB DMA rows)
    wqk = np.ascontiguousarray(
        np.concatenate(
            [wqT.reshape(KT, P, C), wkT.reshape(KT, P, C)], axis=2
        ).reshape(C, 2 * C)
    )
    # wv/wp swizzled to SBUF layout [p, k*C+c_out] (4KB DMA rows)
    wv_sw = np.ascontiguousarray(wvT.reshape(KT, P, C).transpose(1, 0, 2).reshape(P, KT * C))
    wp_sw = np.ascontiguousarray(wpT.reshape(KT, P, C).transpose(1, 0, 2).reshape(P, KT * C))
    bq_pp = np.ascontiguousarray(bq.astype(np.float32).reshape(C // P, P).T)
    # v bias folds into output bias: out = (y' + bv) @ Wp.T + bp
    bpp = (
        Wp.astype(np.float64) @ bv.astype(np.float64) + bp.astype(np.float64)
    ).astype(np.float32)
    bppb = np.broadcast_to(bpp[None, :], (P, C)).copy()
    ones64 = np.ones((P, HD), dtype=f16)
    tri = (np.arange(P)[:, None] <= np.arange(P)[None, :]).astype(f16)  # keep j<=i
    j_idx = np.arange(P)[:, None] + P * np.arange(T // P)[None, :]
    cmask = np.where(j_idx % JD == JD - 1, np.float32(NEG), np.float32(0.0)).astype(
        np.float32
    )

    shared = {
        "wqk": wqk,
        "wv_sw": wv_sw,
        "wp_sw": wp_sw,
        "bq": bq_pp,
        "bppb": bppb,
        "ones64": ones64,
        "ones_row": np.ones((1, P), dtype=f16),
        "bpp_row": bpp[None, :].astype(f16),
        "tri": tri,
        "cmask": cmask,
    }
    in_maps = []
    for b in range(N_CORES):
        m = dict(shared)
        m["xT"] = np.ascontiguousarray(x[b].T).astype(f16)
        in_maps.append(m)
    return in_maps


def kernel(x, Wq, bq, Wk, bk, Wv, bv, Wp, bp):
    from concourse import bass_utils

    x = np.asarray(x, dtype=np.float32)
    if "nc" not in _CACHE:
        _CACHE["nc"] = _build()
    nc = _CACHE["nc"]
    in_maps = _prep_inputs(
        x,
        np.asarray(Wq, np.float32),
        np.asarray(bq, np.float32),
        np.asarray(Wk, np.float32),
        np.asarray(bk, np.float32),
        np.asarray(Wv, np.float32),
        np.asarray(bv, np.float32),
        np.asarray(Wp, np.float32),
        np.asarray(bp, np.float32),
    )
    res = bass_utils.run_bass_kernel_spmd(nc, in_maps, core_ids=list(range(N_CORES)))
    return np.stack(
        [res.results[b]["out"].astype(np.float32) for b in range(N_CORES)], axis=0
    )


# revision 23
# speedup vs baseline: 1.0227x; 1.0227x over previous
"""Causal self-attention (sparse column mask) on 8 Trainium2 NeuronCores.

Problem: B=8, T=1024, C=512, 8 heads (hd=64).
  q/k/v = x @ W{q,k,v}.T + b;  att = softmax(mask(q k^T / 8));  y = att v
  out = y @ Wp.T + bp
Mask: causal lower-triangle, minus every column j with j % 25 == 24.

Sharding: pure data-parallel over batch - core b computes batch element b.

v2 design (fp16 matmul operands, f32 PSUM). Same math as v1; restructured
schedule so the PE stream is dense from ~2us on:
  - k-outer projection start: the first q/k projections consume x/W DMA
    chunks as they arrive instead of waiting for the full tensors.
  - QK tile pairs run concurrently in the PE array (row groups 0/64); AV and
    the ones-matmul denominators run as concurrent column-group pairs.
  - Fine-grained interleave: qk/av/proj/outproj matmuls are woven so the PE
    never has to wait for ACT exp between QK tiles (st PSUM recycling).
  - Output projection r0..3 only needs the first query half -> runs as filler
    during the second attention chunk; r4..7 are pre-accumulated (m=0..2)
    into PSUM banks freed by the last QK phase, so the tail after the final
    AV is just 4 matmuls + evac + small fp16 DMAs.
  - Output written fp16 (host upcasts); per-row-tile DMA on the gpsimd ring.
"""

import numpy as np

B, T, C = 8, 1024, 512
H = 8
HD = C // H
P = 128
JD = 25  # joined dim; column j masked when j % 25 == 24
N_CORES = 8
NEG = -30.0  # added post-scale; exp(-30) flushes to 0 in fp16

_CACHE = {}


def _build():
    import concourse.bass as bass
    import concourse.mybir as mybir
    import concourse.tile as tile
    from concourse import bacc

    f16 = mybir.dt.float16
    f32 = mybir.dt.float32
    AF = mybir.ActivationFunctionType
    ALU = mybir.AluOpType

    nc = bacc.Bacc("TRN2", target_bir_lowering=False, debug=False)

    KT = C // P  # 4 c_in tiles
    MT = C // P  # 4 c_out tiles (= head pairs)
    RT = T // P  # 8 t tiles

    xT = nc.dram_tensor("xT", [C, T], f16, kind="ExternalInput").ap()
    # wq/wk column-interleaved per c_in row (2KB DMA rows); wv/wp pre-swizzled
    # to their SBUF layout (4KB DMA rows, single transfer each).
    wqk = nc.dram_tensor("wqk", [C, 2 * C], f16, kind="ExternalInput").ap()
    wv_d = nc.dram_tensor("wv_sw", [P, KT * C], f16, kind="ExternalInput").ap()
    wp_d = nc.dram_tensor("wp_sw", [P, KT * C], f16, kind="ExternalInput").ap()
    bq = nc.dram_tensor("bq", [P, C // P], f32, kind="ExternalInput").ap()
    bppb = nc.dram_tensor("bppb", [P, C], f32, kind="ExternalInput").ap()
    ones64 = nc.dram_tensor("ones64", [P, HD], f16, kind="ExternalInput").ap()
    ones_row = nc.dram_tensor("ones_row", [1, P], f16, kind="ExternalInput").ap()
    bpp_row = nc.dram_tensor("bpp_row", [1, C], f16, kind="ExternalInput").ap()
    tri = nc.dram_tensor("tri", [P, P], f16, kind="ExternalInput").ap()
    cmask = nc.dram_tensor("cmask", [P, T // P], f32, kind="ExternalInput").ap()
    out = nc.dram_tensor("out", [T, C], f16, kind="ExternalOutput").ap()

    with tile.TileContext(nc) as tc:
        with (
            tc.tile_pool(name="const", bufs=1) as const,
            tc.tile_pool(name="persist", bufs=1) as persist,
            tc.tile_pool(name="es", bufs=16) as es_pool,
            tc.tile_pool(name="rden", bufs=4) as rden_pool,
            tc.tile_pool(name="ot", bufs=4) as ot_pool,
            tc.tile_pool(name="stp", bufs=2, space="PSUM") as stp,
            tc.tile_pool(name="work", bufs=4, space="PSUM") as work,
        ):
            # ---- input DMAs: chunked, ordered by first consumption ----
            def load(shape, dtype, src, tag, eng):
                t = const.tile(shape, dtype, name=tag, tag=tag)
                eng.dma_start(out=t, in_=src)
                return t

            r3 = lambda a: a.rearrange("(a p) n -> p a n", p=P)  # noqa: E731
            # sync (HWDGE) ring, ordered by first consumption: per-k pairs
            # (xT full-row chunk, wq||wk interleaved chunk) - 2KB DMA rows -
            # then the pre-swizzled wv (4KB rows, one transfer).
            # k=0 split small so the very first projection matmul starts ~2us
            # earlier (less data to wait for through the DMA-queue ramp).
            xT_c, wqk_c = [None] * KT, [None] * KT
            x0a = load([P, 1, 512], f16, r3(xT)[:, 0:1, 0:512], "xT0a", nc.sync)
            wq0 = load([P, 1, C], f16, r3(wqk)[:, 0:1, 0:C], "wq0", nc.sync)
            x0b = load([P, 1, 512], f16, r3(xT)[:, 0:1, 512:1024], "xT0b", nc.sync)
            wk0 = load([P, 1, C], f16, r3(wqk)[:, 0:1, C : 2 * C], "wk0", nc.sync)
            for k in range(1, KT):
                xT_c[k] = load([P, 1, T], f16, r3(xT)[:, k : k + 1, :], f"xT{k}", nc.sync)
                wqk_c[k] = load(
                    [P, 1, 2 * C], f16, r3(wqk)[:, k : k + 1, :], f"wqk{k}", nc.sync
                )
            wv_s = load([P, KT * C], f16, wv_d, "wv", nc.sync)
            # small consts on the scalar (ACT) ring: done before the first exp
            tri_s = load([P, P], f16, tri, "tri", nc.scalar)
            cmask_s = load([P, T // P], f32, cmask, "cmask", nc.scalar)
            bq_s = load([P, C // P], f32, bq, "bq", nc.scalar)
            ones64_s = load([P, HD], f16, ones64, "ones64", nc.scalar)
            ones_row_s = load([1, P], f16, ones_row, "ones_row", nc.scalar)
            bpp_row_s = load([1, C], f16, bpp_row, "bpp_row", nc.scalar)
            # late-needed bulk on the gpsimd software-DGE ring
            bppb_s = load([P, C], f32, bppb, "bppb", nc.gpsimd)
            wp_s = load([P, KT * C], f16, wp_d, "wp", nc.gpsimd)

            def xt(k, h):
                if k == 0:
                    return (x0a if h == 0 else x0b)[:, 0, :]
                return xT_c[k][:, 0, 512 * h : 512 * (h + 1)]

            def xts(k, c0, w):  # columns [c0, c0+w) of xT chunk k
                if k == 0:
                    t = x0a if c0 < 512 else x0b
                    c0 = c0 % 512
                    return t[:, 0, c0 : c0 + w]
                return xT_c[k][:, 0, c0 : c0 + w]

            def wq_sl(k, m):
                if k == 0:
                    return wq0[:, 0, P * m : P * (m + 1)]
                return wqk_c[k][:, 0, P * m : P * (m + 1)]

            def wk_sl(k, m):
                if k == 0:
                    return wk0[:, 0, P * m : P * (m + 1)]
                return wqk_c[k][:, 0, C + P * m : C + P * (m + 1)]

            qT_t = [persist.tile([P, T], f16, name=f"qT{m}", tag=f"qT{m}") for m in range(MT)]
            kT_t = [persist.tile([P, T], f16, name=f"kT{m}", tag=f"kT{m}") for m in range(MT)]
            v_t = [persist.tile([P, C], f16, name=f"v{r}", tag=f"v{r}") for r in range(RT)]
            yn_t = [persist.tile([P, T], f16, name=f"yn{m}", tag=f"yn{m}") for m in range(MT)]

            # broadcast lower-triangle tile across both heads of an es tile
            tri_b = bass.AP(
                tensor=tri_s.tensor,
                offset=tri_s.offset,
                ap=[list(tri_s.ap[0]), [0, 2], list(tri_s.ap[1])],
            )

            # ---- emission helpers ----
            def proj_qk_m(m, h):
                """q and k projections for pair m, query half h (k-outer)."""
                qs = work.tile([P, 512], f32, name="qps", tag="wk")
                ks = work.tile([P, 512], f32, name="kps", tag="wk")
                for k in range(KT):
                    nc.tensor.matmul(
                        qs,
                        lhsT=wq_sl(k, m),
                        rhs=xt(k, h),
                        start=(k == 0),
                        stop=(k == KT - 1),
                    )
                    nc.tensor.matmul(
                        ks,
                        lhsT=wk_sl(k, m),
                        rhs=xt(k, h),
                        start=(k == 0),
                        stop=(k == KT - 1),
                    )
                sl = slice(512 * h, 512 * (h + 1))
                nc.vector.tensor_scalar_add(qT_t[m][:, sl], qs, bq_s[:, m : m + 1])
                nc.vector.tensor_copy(kT_t[m][:, sl], ks)

            def proj_v(r):
                pv = work.tile([P, C], f32, name="pv", tag="wk")
                for k in range(KT):
                    nc.tensor.matmul(
                        pv,
                        lhsT=xts(k, P * r, P),
                        rhs=wv_s[:, C * k : C * (k + 1)],
                        start=(k == 0),
                        stop=(k == KT - 1),
                    )
                nc.vector.tensor_copy(v_t[r], pv)

            es_t = {}

            def qk2(ic, p, jp):
                """two QK key-tiles (J=2jp, 2jp+1) + exp (+ causal tri)."""
                for J in (2 * jp, 2 * jp + 1):
                    if J >= 4 * (ic + 1):
                        continue
                    i0 = max(512 * ic, P * J)
                    w = 512 * (ic + 1) - i0
                    st = stp.tile([P, 2, 512], f32, name="st", tag="st")
                    for h in range(2):
                        nc.tensor.matmul(
                            st[:, h, :w],
                            lhsT=kT_t[p][64 * h : 64 * (h + 1), P * J : P * (J + 1)],
                            rhs=qT_t[p][64 * h : 64 * (h + 1), i0 : i0 + w],
                            start=True,
                            stop=True,
                            tile_position=(64 * h, 0),
                        )
                    es = es_pool.tile([P, 2, 512], f16, name="es", tag="es")
                    es_t[(ic, p, J)] = es
                    nc.scalar.activation(
                        es[:, :, :w],
                        st[:, :, :w],
                        AF.Exp,
                        bias=cmask_s[:, J : J + 1],
                        scale=0.125,
                    )
                    if P * J >= 512 * ic:  # diagonal: zero the causal triangle
                        nc.vector.tensor_tensor(
                            out=es[:, :, :P], in0=es[:, :, :P], in1=tri_b, op=ALU.mult
                        )

            av_ps = {}

            def av2(ic, p, jp):
                """two AV+den key-tiles for (ic, p)."""
                nJ = 4 * (ic + 1)
                if jp == 0 and (ic, p) not in av_ps:
                    av_ps[(ic, p)] = (
                        work.tile([P, 512], f32, name="av", tag="wk"),
                        work.tile([P, 512], f32, name="den", tag="wk"),
                    )
                av, den = av_ps[(ic, p)]
                for J in (2 * jp, 2 * jp + 1):
                    if J >= nJ:
                        continue
                    i0 = max(512 * ic, P * J)
                    w = 512 * (ic + 1) - i0
                    io = i0 - 512 * ic
                    first, last = J == 0, J == nJ - 1
                    es = es_t.pop((ic, p, J))
                    for h in range(2):
                        nc.tensor.matmul(
                            av[64 * h : 64 * (h + 1), io : io + w],
                            lhsT=v_t[J][:, P * p + 64 * h : P * p + 64 * (h + 1)],
                            rhs=es[:, h, :w],
                            start=first,
                            stop=last,
                            tile_position=(0, 64 * h),
                        )
                        nc.tensor.matmul(
                            den[64 * h : 64 * (h + 1), io : io + w],
                            lhsT=ones64_s,
                            rhs=es[:, h, :w],
                            start=first,
                            stop=last,
                            tile_position=(0, 64 * h),
                        )

            def rden_mul(ic, p):
                av, den = av_ps.pop((ic, p))
                rden = rden_pool.tile([P, 512], f32, name="rden", tag="rden")
                nc.vector.reciprocal_approx_fast(out=rden, in_=den)
                nc.vector.tensor_mul(yn_t[p][:, 512 * ic : 512 * (ic + 1)], av, rden)

            def av4(ic, p):
                av2(ic, p, 0)
                av2(ic, p, 1)
                rden_mul(ic, p)

            def op_emit(r, po, m0, m1, bias_mm=False):
                """outproj matmuls m0..m1 for row-tile r into po."""
                for m in range(m0, m1 + 1):
                    nc.tensor.matmul(
                        po,
                        lhsT=yn_t[m][:, P * r : P * (r + 1)],
                        rhs=wp_s[:, C * m : C * (m + 1)],
                        start=(m == 0),
                        stop=(m == MT - 1),
                    )
                if bias_mm:  # rank-1 bias add folded into the PSUM group
                    nc.tensor.matmul(
                        po, lhsT=ones_row_s, rhs=bpp_row_s, start=False, stop=False
                    )

            def op_dma(r, ot):
                nc.sync.dma_start(out=out[P * r : P * (r + 1), :], in_=ot)

            def op_full(r):
                po = work.tile([P, C], f32, name=f"po{r}", tag="wk")
                op_emit(r, po, 0, MT - 1)
                ot = ot_pool.tile([P, C], f16, name="ot", tag="ot")
                nc.vector.tensor_tensor(out=ot, in0=po, in1=bppb_s, op=ALU.add)
                op_dma(r, ot)

            # ================= emission schedule =================
            # A: projections h0 woven with the first QK tiles
            proj_qk_m(0, 0)
            qk2(0, 0, 0)
            proj_qk_m(1, 0)
            qk2(0, 0, 1)
            proj_qk_m(2, 0)
            qk2(0, 1, 0)
            proj_qk_m(3, 0)
            qk2(0, 1, 1)
            # B: ic=0 attention + v + h1 projections
            proj_v(0)
            proj_v(1)
            qk2(0, 2, 0)
            proj_v(2)
            proj_v(3)
            qk2(0, 2, 1)
            av4(0, 0)
            qk2(0, 3, 0)
            proj_qk_m(0, 1)
            qk2(0, 3, 1)
            av4(0, 1)
            proj_qk_m(1, 1)
            av4(0, 2)
            proj_qk_m(2, 1)
            av4(0, 3)
            proj_qk_m(3, 1)
            # C: ic=1 attention; av lags qk by one phase; outproj r0..3 and the
            # remaining v tiles fill the gaps.
            qk2(1, 0, 0)
            op_full(0)
            qk2(1, 0, 1)
            op_full(1)
            qk2(1, 0, 2)
            proj_v(4)
            proj_v(5)
            qk2(1, 0, 3)
            proj_v(6)
            proj_v(7)
            for p in (1, 2, 3):
                for jp in range(4):
                    qk2(1, p, jp)
                    av2(1, p - 1, jp)
                    if p == 1 and jp == 0:
                        op_full(2)
                    if p == 1 and jp == 1:
                        op_full(3)
                rden_mul(1, p - 1)
            # tail: av(1,3) uses the work pool (av2 allocates); outproj r4/r5
            # pre-accumulate into one st-pool tile pair (their evacs overlap
            # the last AV tiles), while r6/r7 - the true end of the critical
            # path - get independent work-pool banks so their final matmuls
            # and ACT/DVE evacuations run fully in parallel. The (1,3)
            # normalization runs in 128-column chunks that finalize as soon
            # as their last contributing key-tile lands.
            av2(1, 3, 0)
            po45 = stp.tile([P, 2, 512], f32, name="po45", tag="st")
            po_hi = {4: po45[:, 0, :], 5: po45[:, 1, :]}
            for r in (6, 7):
                po_hi[r] = work.tile([P, C], f32, name=f"po{r}", tag="wk")
            op_emit(6, po_hi[6], 0, 2, bias_mm=True)
            op_emit(7, po_hi[7], 0, 2, bias_mm=True)
            av2(1, 3, 1)
            op_emit(4, po_hi[4], 0, 2, bias_mm=True)
            op_emit(5, po_hi[5], 0, 2, bias_mm=True)
            av2(1, 3, 2)

            av13, den13 = av_ps[(1, 3)]
            rden13 = rden_pool.tile([P, 512], f32, name="rden", tag="rden")

            def norm_chunk(c0, c1):  # columns [128*c0, 128*c1) of the ic=1 half
                sl = slice(P * c0, P * c1)
                nc.vector.reciprocal_approx_fast(out=rden13[:, sl], in_=den13[:, sl])
                nc.vector.tensor_mul(
                    yn_t[3][:, 512 + P * c0 : 512 + P * c1], av13[:, sl], rden13[:, sl]
                )

            norm_chunk(0, 2)  # queries 512..767: all key-tiles J0..J5 done
            op_emit(4, po_hi[4], 3, 3)
            op_emit(5, po_hi[5], 3, 3)
            av2(1, 3, 3)
            norm_chunk(2, 4)
            op_emit(6, po_hi[6], 3, 3)
            op_emit(7, po_hi[7], 3, 3)
            # evacuations split across ACT (idle by now) and DVE
            for r in range(4, 8):
                ot = ot_pool.tile([P, C], f16, name="ot", tag="ot")
                if r % 2 == 0:
                    nc.scalar.activation(ot, po_hi[r], AF.Copy)
                else:
                    nc.vector.tensor_copy(ot, po_hi[r])
                op_dma(r, ot)

    nc.compile()
    return nc


def _prep_inputs(x, Wq, bq, Wk, bk, Wv, bv, Wp, bp):
    """Host-side prep: transposes, bias folding, mask tables. Returns in_maps."""
    f16 = np.float16
    KT = C // P
    wqT = np.ascontiguousarray(Wq.T).astype(f16)
    wkT = np.ascontiguousarray(Wk.T).astype(f16)
    wvT = np.ascontiguousarray(Wv.T).astype(f16)
    wpT = np.ascontiguousarray(Wp.T).astype(f16)
    # wq/wk column-interleaved per c_in (2KB DMA rows)
    wqk = np.ascontiguousarray(
        np.concatenate(
            [wqT.reshape(KT, P, C), wkT.reshape(KT, P, C)], axis=2
        ).reshape(C, 2 * C)
    )
    # wv/wp swizzled to SBUF layout [p, k*C+c_out] (4KB DMA rows)
    wv_sw = np.ascontiguousarray(wvT.reshape(KT, P, C).transpose(1, 0, 2).reshape(P, KT * C))
    wp_sw = np.ascontiguousarray(wpT.reshape(KT, P, C).transpose(1, 0, 2).reshape(P, KT * C))
    bq_pp = np.ascontiguousarray(bq.astype(np.float32).reshape(C // P, P).T)
    # v bias folds into output bias: out = (y' + bv) @ Wp.T + bp
    bpp = (
        Wp.astype(np.float64) @ bv.astype(np.float64) + bp.astype(np.float64)
    ).astype(np.float32)
    bppb = np.broadcast_to(bpp[None, :], (P, C)).copy()
    ones64 = np.ones((P, HD), dtype=f16)
    tri = (np.arange(P)[:, None] <= np.arange(P)[None, :]).astype(f16)  # keep j<=i
    j_idx = np.arange(P)[:, None] + P * np.arange(T // P)[None, :]
    cmask = np.where(j_idx % JD == JD - 1, np.float32(NEG), np.float32(0.0)).astype(
        np.float32
    )

    shared = {
        "wqk": wqk,
        "wv_sw": wv_sw,
        "wp_sw": wp_sw,
        "bq": bq_pp,
        "bppb": bppb,
        "ones64": ones64,
        "ones_row": np.ones((1, P), dtype=f16),
        "bpp_row": bpp[None, :].astype(f16),
        "tri": tri,
        "cmask": cmask,
    }
    in_maps = []
    for b in range(N_CORES):
        m = dict(shared)
        m["xT"] = np.ascontiguousarray(x[b].T).astype(f16)
        in_maps.append(m)
    return in_maps


def kernel(x, Wq, bq, Wk, bk, Wv, bv, Wp, bp):
    from concourse import bass_utils

    x = np.asarray(x, dtype=np.float32)
    if "nc" not in _CACHE:
        _CACHE["nc"] = _build()
    nc = _CACHE["nc"]
    in_maps = _prep_inputs(
        x,
        np.asarray(Wq, np.float32),
        np.asarray(bq, np.float32),
        np.asarray(Wk, np.float32),
        np.asarray(bk, np.float32),
        np.asarray(Wv, np.float32),
        np.asarray(bv, np.float32),
        np.asarray(Wp, np.float32),
        np.asarray(bp, np.float32),
    )
    res = bass_utils.run_bass_kernel_spmd(nc, in_maps, core_ids=list(range(N_CORES)))
    return np.stack(
        [res.results[b]["out"].astype(np.float32) for b in range(N_CORES)], axis=0
    )
